# revision 4
# baseline (speedup 1.0000x reference)
"""Sparse 3-layer voxel-graph convolution on 8 Trainium2 NeuronCores.

The 27-neighborhood graph over a 128^3 voxel grid at ~9.5% occupancy is
below the percolation threshold: it splits into ~31K small connected
components (largest ~2.4K points).  Components are bin-packed into 8
closed per-core subgraphs -> zero halo, no collectives, each core runs
the full 3-layer network on its own points.

Within a core the convolution is computed sparsely (only ~3.44 of 27
neighbor slots are valid on average):
  - per layer, one dma_gather per entry-slab pulls h[src] rows (256B)
    from HBM into SBUF, entries grouped by kernel offset k
  - per 128-entry chunk: PE transpose (-> channels on partitions), PE
    matmul with W[k] -> Z rows in PSUM, copy to an SBUF Z slab
  - k=13 (center) entries are the identity map: their Z rows initialize
    the accumulator with a contiguous DMA write (bias fused via DVE add)
  - all other k blocks accumulate with dma_scatter_add (SWDGE CCE add);
    within a k block destination rows are distinct, and Tile's WAW
    tracking serializes the scatter ops against each other
  - relu pass: acc -> h_next in HBM

Host-side preprocessing (pure numpy) computes the partition, local
index remapping and the int16 wrapped index buffers that the Q7 SWDGE
consumes.  Padding entries point at a trash row past the real points.
"""

import sys
import numpy as np

for _p in ("/opt/trn_rl_repo",):
    if _p not in sys.path:
        sys.path.insert(0, _p)

C = 64
K = 27
KC = 13  # center offset (identity)
NCORES = 8
P = 128
SLAB13 = 4096  # entries per gather for the k=13 block


def _round_up(x, m):
    return (x + m - 1) // m * m


# ---------------------------------------------------------------- host prep

def _partition(nbr):
    """Connected components -> greedy bin-pack into NCORES groups."""
    import scipy.sparse as sp
    from scipy.sparse.csgraph import connected_components

    N = nbr.shape[0]
    ks = [k for k in range(K) if k != KC]
    dsts, srcs = [], []
    for k in ks:
        d = np.nonzero(nbr[:, k] >= 0)[0]
        dsts.append(d)
        srcs.append(nbr[d, k].astype(np.int64))
    rows = np.concatenate(dsts)
    cols = np.concatenate(srcs)
    g = sp.csr_matrix(
        (np.ones(len(rows), np.int8), (rows, cols)), shape=(N, N)
    )
    _, comp = connected_components(g, directed=False)
    ncomp = comp.max() + 1
    sizes = np.bincount(comp, minlength=ncomp)
    order = np.argsort(sizes)[::-1]
    load = np.zeros(NCORES, np.int64)
    comp_core = np.empty(ncomp, np.int32)
    for cid in order:
        c = int(np.argmin(load))
        comp_core[cid] = c
        load[c] += sizes[cid]
    core_of = comp_core[comp]
    return core_of, ks, dsts, srcs


def _prep(nbr):
    """Full host-side plan. Returns a dict of static sizes + per-core data."""
    N = nbr.shape[0]
    core_of, ks, dsts, srcs = _partition(nbr)

    perm = [np.nonzero(core_of == c)[0] for c in range(NCORES)]
    Nc = np.array([len(p) for p in perm], np.int64)
    local = np.empty(N, np.int64)
    for c in range(NCORES):
        local[perm[c]] = np.arange(Nc[c])

    Nc_pad = _round_up(int(Nc.max()), P)
    Ncap = Nc_pad + P
    trash = Nc_pad  # local trash row

    # per-(core, k) local entry lists
    ent = {}
    Ek = np.zeros((NCORES, len(ks)), np.int64)
    for j in range(len(ks)):
        d, s = dsts[j], srcs[j]
        cd = core_of[d]
        for c in range(NCORES):
            m = cd == c
            ent[(c, j)] = (local[s[m]], local[d[m]])
            Ek[c, j] = int(m.sum())
    EKP = _round_up(int(Ek.max()), P)  # uniform padded sparse-block size

    # global entry layout: [k13 rows 0..Nc_pad) | 26 sparse blocks of EKP]
    Etot = Nc_pad + len(ks) * EKP
    gsrc = np.full((NCORES, Etot), trash, np.int64)
    sdst = np.full((NCORES, Etot), trash, np.int64)
    for c in range(NCORES):
        n = int(Nc[c])
        gsrc[c, :n] = np.arange(n)
        for j in range(len(ks)):
            off = Nc_pad + j * EKP
            s_loc, d_loc = ent[(c, j)]
            gsrc[c, off:off + len(s_loc)] = s_loc
            sdst[c, off:off + len(d_loc)] = d_loc

    def wrap(a):  # [Etot] int -> [128, Etot//16] int16 (wrapped + replicated)
        w = a.reshape(-1, 16).T.astype(np.int16)
        return np.tile(w, (P // 16, 1)).copy()

    gidx = np.stack([wrap(gsrc[c]) for c in range(NCORES)])
    sidx = np.stack([wrap(sdst[c]) for c in range(NCORES)])

    return dict(
        N=N, perm=perm, Nc=Nc, Nc_pad=Nc_pad, Ncap=Ncap, EKP=EKP,
        Etot=Etot, ks=ks, gidx=gidx, sidx=sidx,
    )


# ------------------------------------------------------------- bass program

def _build_program(Ncap, Nc_pad, Etot, EKP, ks):
    from contextlib import ExitStack
    import concourse.bacc as bacc
    import concourse.mybir as mybir
    from concourse import tile

    f32 = mybir.dt.float32
    i16 = mybir.dt.int16

    nc = bacc.Bacc("TRN2", target_bir_lowering=False, debug=False)

    feats_d = nc.dram_tensor("feats", [Ncap, C], f32, kind="ExternalInput")
    w_d = [
        nc.dram_tensor("w1", [C, K * C], f32, kind="ExternalInput"),
        nc.dram_tensor("w2", [C, K * C], f32, kind="ExternalInput"),
        nc.dram_tensor("w3", [C, K * 2 * C], f32, kind="ExternalInput"),
    ]
    b_d = [
        nc.dram_tensor("b1", [P, C], f32, kind="ExternalInput"),
        nc.dram_tensor("b2", [P, C], f32, kind="ExternalInput"),
        nc.dram_tensor("b3", [P, 2 * C], f32, kind="ExternalInput"),
    ]
    gidx_d = nc.dram_tensor("gidx", [P, Etot // 16], i16, kind="ExternalInput")
    sidx_d = nc.dram_tensor("sidx", [P, Etot // 16], i16, kind="ExternalInput")
    ident_d = nc.dram_tensor("ident", [P, P], f32, kind="ExternalInput")
    out_d = nc.dram_tensor("out", [Ncap, 2 * C], f32, kind="ExternalOutput")

    # slab plan: (entry_offset, n_entries, k) — k13 slabs first, then one
    # slab per sparse k block
    slabs = []
    for s0 in range(0, Nc_pad, SLAB13):
        slabs.append((s0, min(SLAB13, Nc_pad - s0), KC))
    for j, k in enumerate(ks):
        slabs.append((Nc_pad + j * EKP, EKP, k))
    maxT = max(n for _, n, _ in slabs) // P

    with ExitStack() as ctx:
        tc = ctx.enter_context(tile.TileContext(nc))
        const = ctx.enter_context(tc.tile_pool(name="const", bufs=1))
        dram = ctx.enter_context(tc.tile_pool(name="dram", bufs=1, space="DRAM"))
        gpool = ctx.enter_context(tc.tile_pool(name="g", bufs=3))
        zpool = ctx.enter_context(tc.tile_pool(name="z", bufs=3))
        gtpool = ctx.enter_context(tc.tile_pool(name="gt", bufs=6))
        ptpool = ctx.enter_context(tc.tile_pool(name="pt", bufs=3, space="PSUM"))
        pzpool = ctx.enter_context(tc.tile_pool(name="pz", bufs=3, space="PSUM"))
        rpool = ctx.enter_context(tc.tile_pool(name="r", bufs=3))

        # persistent SBUF state
        gidx = const.tile([P, Etot // 16], i16)
        nc.sync.dma_start(gidx[:], gidx_d.ap()[:, :])
        sidx = const.tile([P, Etot // 16], i16)
        nc.sync.dma_start(sidx[:], sidx_d.ap()[:, :])
        ident = const.tile([P, P], f32)
        nc.sync.dma_start(ident[:], ident_d.ap()[:, :])
        wt, bt = [], []
        for L in range(3):
            co = 2 * C if L == 2 else C
            w = const.tile([C, K * co], f32, tag=f"w{L}")
            nc.sync.dma_start(w[:], w_d[L].ap()[:, :])
            wt.append(w)
            b = const.tile([P, co], f32, tag=f"b{L}")
            nc.sync.dma_start(b[:], b_d[L].ap()[:, :])
            bt.append(b)
        zero = const.tile([P, 2 * C], f32)
        nc.vector.memset(zero[:], 0.0)

        acc = dram.tile([Ncap, C], f32)
        h1 = dram.tile([Ncap, C], f32)
        h2 = dram.tile([Ncap, C], f32)

        n_chunks = Ncap // P
        for L in range(3):
            co = 2 * C if L == 2 else C
            hsrc = [feats_d.ap(), h1[:, :], h2[:, :]][L]
            tgt = out_d.ap() if L == 2 else acc[:, :]
            tgt_w = tgt.rearrange("(t p) c -> p t c", p=P)

            # zero the trash chunk (keeps pad-entry arithmetic finite)
            nc.sync.dma_start(
                tgt_w[:, Nc_pad // P: Nc_pad // P + 1, :],
                zero[:, :co].rearrange("p (t c) -> p t c", t=1),
            )

            for (e0, n, k) in slabs:
                T = n // P
                g = gpool.tile([P, maxT, C], f32, tag="g")
                nc.gpsimd.dma_gather(
                    g[:, :T, :], hsrc[:, :],
                    gidx[:, e0 // 16: (e0 + n) // 16], n, n, C,
                    single_packet=False,
                )
                z = zpool.tile([P, maxT, co], f32, tag="z")
                for t in range(T):
                    pt = ptpool.tile([C, P], f32, tag="pt")
                    nc.tensor.transpose(pt[:], g[:, t, :], ident[:])
                    gts = gtpool.tile([C, P], f32, tag="gt")
                    nc.scalar.copy(gts[:], pt[:])
                    zp = pzpool.tile([P, co], f32, tag="pz")
                    nc.tensor.matmul(
                        zp[:], gts[:], wt[L][:, k * co: (k + 1) * co],
                        start=True, stop=True,
                    )
                    if k == KC:
                        nc.vector.tensor_add(z[:, t, :], zp[:], bt[L][:, :])
                    else:
                        nc.vector.tensor_copy(z[:, t, :], zp[:])
                if k == KC:
                    nc.sync.dma_start(
                        tgt_w[:, e0 // P: e0 // P + T, :], z[:, :T, :]
                    )
                else:
                    nc.gpsimd.dma_scatter_add(
                        tgt[:, :], z[:, :T, :],
                        sidx[:, e0 // 16: (e0 + n) // 16], n, n, co,
                        single_packet=False,
                    )

            if L < 2:  # relu: acc -> h_{L+1} (covers trash chunk too)
                hn = [h1, h2][L][:, :]
                hn_w = hn.rearrange("(t p) c -> p t c", p=P)
                acc_w = acc[:, :].rearrange("(t p) c -> p t c", p=P)
                RT = 16
                for c0 in range(0, n_chunks, RT):
                    T = min(RT, n_chunks - c0)
                    r = rpool.tile([P, RT, C], f32, tag="r")
                    nc.sync.dma_start(r[:, :T, :], acc_w[:, c0:c0 + T, :])
                    nc.vector.tensor_scalar_max(r[:, :T, :], r[:, :T, :], 0.0)
                    nc.sync.dma_start(hn_w[:, c0:c0 + T, :], r[:, :T, :])

    nc.compile()
    return nc


# ------------------------------------------------------------------ driver

_CACHE = {}


def _get_compiled(nbr_bytes_key, nbr):
    if nbr_bytes_key not in _CACHE:
        plan = _prep(nbr)
        nc = _build_program(
            plan["Ncap"], plan["Nc_pad"], plan["Etot"], plan["EKP"], plan["ks"]
        )
        _CACHE[nbr_bytes_key] = (plan, nc)
    return _CACHE[nbr_bytes_key]


def _pack_inputs(plan, feats, W1, b1, W2, b2, W3, b3):
    """Build the 8 per-core input maps."""
    Ncap, Ncs, perm = plan["Ncap"], plan["Nc"], plan["perm"]
    ws = []
    for W in (W1, W2, W3):
        co = W.shape[-1]
        ws.append(np.ascontiguousarray(
            W.transpose(1, 0, 2).reshape(C, K * co)).astype(np.float32))
    bs = [np.tile(b.reshape(1, -1), (P, 1)).astype(np.float32)
          for b in (b1, b2, b3)]
    ident = np.eye(P, dtype=np.float32)
    in_maps = []
    for c in range(NCORES):
        fc = np.zeros((Ncap, C), np.float32)
        fc[: Ncs[c]] = feats[perm[c]]
        in_maps.append({
            "feats": fc,
            "w1": ws[0], "w2": ws[1], "w3": ws[2],
            "b1": bs[0], "b2": bs[1], "b3": bs[2],
            "gidx": plan["gidx"][c], "sidx": plan["sidx"][c],
            "ident": ident,
        })
    return in_maps


def kernel(feats, W1, b1, W2, b2, W3, b3, neighbor_idx):
    feats = np.asarray(feats, np.float32)
    W1 = np.asarray(W1, np.float32)
    W2 = np.asarray(W2, np.float32)
    W3 = np.asarray(W3, np.float32)
    b1 = np.asarray(b1, np.float32)
    b2 = np.asarray(b2, np.float32)
    b3 = np.asarray(b3, np.float32)
    nbr = np.asarray(neighbor_idx)

    key = (nbr.shape, hash(nbr.tobytes()))
    plan, nc = _get_compiled(key, nbr)
    in_maps = _pack_inputs(plan, feats, W1, b1, W2, b2, W3, b3)

    from concourse.bass_utils import run_bass_kernel_spmd
    res = run_bass_kernel_spmd(nc, in_maps, core_ids=list(range(NCORES)))

    N = plan["N"]
    out = np.empty((N, 2 * C), np.float32)
    for c in range(NCORES):
        out[plan["perm"][c]] = np.asarray(res.results[c]["out"])[: plan["Nc"][c]]
    return out
